# revision 6
# baseline (speedup 1.0000x reference)
"""Chamfer loss kernel for Trainium2 (8 NeuronCores).

Problem: preds [8, 8192, 3], gts [8, 8192, 3] (f32).
  P[b,n,m] = ||gts[b,n] - preds[b,m]||^2
  loss = sum_{b,m} min_n P[b,n,m] + sum_{b,n} min_m P[b,n,m]

Sharding: data-parallel over batch B — one batch element per core; the host
sums the 8 per-core partial losses.

Per-core algorithm (two symmetric passes; pass A shown, pass B swaps roles):
  min_m P[n,m] = xx[n] + min_m ( -2 g[n]·p[m] + yy[m] )
The bracketed term is computed on TensorE as a K=15 bf16 matmul that is
exact to ~f32: (-2g) and p are split into bf16 hi+lo (all 4 cross terms
kept), and yy gets a 3-way bf16 split against 'ones' rows.  xx is added on
the host after the min (it is constant over the min axis).

The min-reduction drains PSUM through both PSUM-capable engines:
  - DVE reduces some 2-bank chunks directly (1 elem/lane/cyc),
  - ScalarE copies the rest to SBUF as bf16, which DVE then min-reduces via
    tensor_scalar(op0=min, op1=min, accum_out=...) at the fast packed mode.

This walrus build only accepts ONE sync-wait per instruction, so the BIR
json is post-processed to hoist extra waits onto EventSemaphore carriers.
"""

import json

import numpy as np
import ml_dtypes

BF16 = ml_dtypes.bfloat16

B, N, M, D = 8, 8192, 8192, 3
P = 128           # partitions per n-tile
NT = N // P       # 64 n-tiles
CH = 512          # one PSUM bank of f32
WIDE = 1024       # DVE/ACT operate on 2-bank chunks
NW = M // WIDE    # 8 wide chunks per n-tile
K = 18            # matmul contraction rows (split-fp32 encoding)
N_DIRECT = 3      # wide chunks per n-tile reduced directly from PSUM by DVE
NCORES = 8
BIG = 3.0e38


def _split_bf16(x):
    hi = x.astype(BF16)
    lo = (x - hi.astype(np.float32)).astype(BF16)
    return hi, lo


def _split3_bf16(x):
    """x (f32) -> three bf16 arrays summing to ~x (residual ~2^-27 rel)."""
    hi = x.astype(BF16)
    r1 = x - hi.astype(np.float32)
    mid = r1.astype(BF16)
    r2 = r1 - mid.astype(np.float32)
    lo = r2.astype(BF16)
    return hi, mid, lo


def _build_pass(a_pts, b_pts):
    """lhsT [K,N] bf16, rhs [K,M] bf16 with
    lhsT.T @ rhs ~= ||a||^2 - 2 a·b + ||b||^2  (full squared distance)."""
    a = a_pts.astype(np.float32)
    b = b_pts.astype(np.float32)
    t = -2.0 * a
    t_hi, t_lo = _split_bf16(t)
    p_hi, p_lo = _split_bf16(b)
    yy = (b * b).sum(-1, dtype=np.float32)
    yy_hi, yy_mid, yy_lo = _split3_bf16(yy)
    xx = (a * a).sum(-1, dtype=np.float32)
    xx_hi, xx_mid, xx_lo = _split3_bf16(xx)
    ones_a = np.ones((a.shape[0],), dtype=BF16)
    ones_b = np.ones((b.shape[0],), dtype=BF16)

    lhsT = np.stack(
        [t_hi[:, 0], t_hi[:, 1], t_hi[:, 2],
         t_hi[:, 0], t_hi[:, 1], t_hi[:, 2],
         t_lo[:, 0], t_lo[:, 1], t_lo[:, 2],
         t_lo[:, 0], t_lo[:, 1], t_lo[:, 2],
         ones_a, ones_a, ones_a,
         xx_hi, xx_mid, xx_lo]
    )
    rhs = np.stack(
        [p_hi[:, 0], p_hi[:, 1], p_hi[:, 2],
         p_lo[:, 0], p_lo[:, 1], p_lo[:, 2],
         p_hi[:, 0], p_hi[:, 1], p_hi[:, 2],
         p_lo[:, 0], p_lo[:, 1], p_lo[:, 2],
         yy_hi, yy_mid, yy_lo,
         ones_b, ones_b, ones_b]
    )
    return lhsT, rhs


MAX_WAITS = 1


def _split_waits_json(raw: bytes) -> bytes:
    """Hoist extra sync-waits onto EventSemaphore carriers (this walrus build
    rejects instructions with more than one wait)."""
    d = json.loads(raw)
    for f in d["functions"]:
        for blk in f["blocks"]:
            insts = blk.get("instructions")
            if not insts:
                continue
            new = []
            changed = False
            for inst in insts:
                si = inst.get("sync_info")
                waits = (si or {}).get("on_wait") or []
                if len(waits) > MAX_WAITS:
                    extra = waits[:-MAX_WAITS]
                    keep = waits[-MAX_WAITS:]
                    for k, w in enumerate(extra):
                        new.append({
                            "debug": inst.get("debug", 0),
                            "engine": inst["engine"],
                            "ins": [], "outs": [],
                            "name": f"{inst['name']}_sw{k}",
                            "opcode": "EventSemaphore",
                            "sync_info": {"on_wait": [w], "on_update": []},
                        })
                    si["on_wait"] = keep
                    changed = True
                new.append(inst)
            if changed:
                blk["instructions"] = new
    return json.dumps(d).encode()


def _build_nc():
    import concourse.bass as bass
    import concourse.tile as tile
    import concourse.mybir as mybir

    f32 = mybir.dt.float32
    bf16 = mybir.dt.bfloat16
    MIN = mybir.AluOpType.min
    X = mybir.AxisListType.X
    NCOPY = NW - N_DIRECT          # wide chunks routed via ACT->SBUF bf16
    SW = NCOPY * WIDE              # width of the bf16 staging tile

    nc = bass.Bass()
    dram = {}
    for nm in ("lA", "rA", "lB", "rB"):
        dram[nm] = nc.dram_tensor(nm, [K, 8192], bf16, kind="ExternalInput")
    out = nc.dram_tensor("out", [P, 2 * NT], f32, kind="ExternalOutput")

    with tile.TileContext(nc) as tc:
        with (
            tc.tile_pool(name="const", bufs=1) as cpool,
            tc.tile_pool(name="psum", bufs=4, space="PSUM") as psum,
            tc.tile_pool(name="stage", bufs=2) as stpool,
            tc.tile_pool(name="parts", bufs=2) as partpool,
        ):
            w = {}
            r = {}
            for nm in ("A", "B"):
                wt = cpool.tile([K, 8192], bf16, name=f"w{nm}", tag=f"w{nm}")
                rt = cpool.tile([K, 8192], bf16, name=f"r{nm}", tag=f"r{nm}")
                nc.sync.dma_start(wt[:, :], dram["l" + nm][:, :])
                nc.sync.dma_start(rt[:, :], dram["r" + nm][:, :])
                w[nm], r[nm] = wt, rt
            minv = cpool.tile([P, 2 * NT], f32, name="minv", tag="minv")

            for pi, nm in enumerate(("A", "B")):
                wt, rt = w[nm], r[nm]
                for i in range(NT):
                    lhsT = wt[:, P * i:P * (i + 1)]
                    parts = partpool.tile([P, N_DIRECT + 1], f32,
                                          name="parts", tag="parts")
                    s5 = stpool.tile([P, SW], bf16, name="s5", tag="s5")
                    ncopied = 0
                    for k in range(NW):
                        W = psum.tile([P, WIDE], f32, name="W", tag="W")
                        for h in range(2):
                            nc.tensor.matmul(
                                W[:, CH * h:CH * (h + 1)],
                                lhsT,
                                rt[:, WIDE * k + CH * h:WIDE * k + CH * (h + 1)],
                                start=True, stop=True,
                            )
                        # interleave direct reductions among the copies
                        if k % 3 == 2 and ncopied < NCOPY or ncopied >= NCOPY:
                            nc.vector.tensor_reduce(
                                parts[:, k - ncopied:k - ncopied + 1], W[:],
                                axis=X, op=MIN,
                            )
                        else:
                            nc.scalar.copy(
                                s5[:, ncopied * WIDE:(ncopied + 1) * WIDE], W[:])
                            ncopied += 1
                    nc.vector.tensor_scalar(
                        s5[:], s5[:], BIG, None,
                        op0=MIN, op1=MIN,
                        accum_out=parts[:, N_DIRECT:N_DIRECT + 1],
                    )
                    nc.vector.tensor_reduce(
                        minv[:, pi * NT + i:pi * NT + i + 1], parts[:],
                        axis=X, op=MIN,
                    )
            nc.sync.dma_start(out[:, :], minv[:])

    orig = nc.to_json_bytes
    nc.to_json_bytes = lambda: _split_waits_json(orig())
    return nc


_LAST_RESULTS = None


def _prepare_in_maps(preds, gts):
    in_maps = []
    for b in range(B):
        lA, rA = _build_pass(gts[b], preds[b])     # min over preds per gt
        lB, rB = _build_pass(preds[b], gts[b])     # min over gts per pred
        in_maps.append({"lA": lA, "rA": rA, "lB": lB, "rB": rB})
    return in_maps


def kernel(preds, gts, _trace=False):
    from concourse.bass_utils import run_bass_kernel_spmd

    global _LAST_RESULTS
    preds = np.asarray(preds)
    gts = np.asarray(gts)
    assert preds.shape == (B, M, D) and gts.shape == (B, N, D)

    in_maps = _prepare_in_maps(preds, gts)
    nc = _build_nc()
    res = run_bass_kernel_spmd(
        nc, in_maps, core_ids=list(range(NCORES)), trace=_trace,
    )
    _LAST_RESULTS = res

    total = 0.0
    for b in range(B):
        total += res.results[b]["out"].astype(np.float64).sum()
    return np.asarray(total, dtype=np.float32)


# ----------------------------------------------------------------------------
# Benchmark support (test-only): build the jitted sharded executable once and
# re-invoke it, so per-call wall time ~= dispatch overhead + NEFF exec time.
# ----------------------------------------------------------------------------

def _make_runner(nc, in_maps):
    import jax
    import jax.numpy as jnp
    import concourse.mybir as mybir
    from concourse import bass2jax
    from jax.experimental.shard_map import shard_map
    from jax.sharding import Mesh, PartitionSpec

    bass2jax.install_neuronx_cc_hook()
    n_cores = len(in_maps)

    partition_name = nc.partition_id_tensor.name if nc.partition_id_tensor else None
    in_names, out_names, out_avals, zero_outs = [], [], [], []
    for alloc in nc.m.functions[0].allocations:
        if not isinstance(alloc, mybir.MemoryLocationSet):
            continue
        name = alloc.memorylocations[0].name
        if alloc.kind == "ExternalInput":
            if name != partition_name:
                in_names.append(name)
        elif alloc.kind == "ExternalOutput":
            shape = tuple(alloc.tensor_shape)
            dtype = mybir.dt.np(alloc.dtype)
            out_names.append(name)
            out_avals.append(jax.core.ShapedArray(shape, dtype))
            zero_outs.append(np.zeros(shape, dtype))
    n_params = len(in_names)
    n_outs = len(out_avals)
    in_names = in_names + out_names
    if partition_name is not None:
        in_names.append(partition_name)
    donate = tuple(range(n_params, n_params + n_outs))

    def _body(*args):
        operands = list(args)
        if partition_name is not None:
            operands.append(bass2jax.partition_id_tensor())
        outs = bass2jax._bass_exec_p.bind(
            *operands,
            out_avals=tuple(out_avals),
            in_names=tuple(in_names),
            out_names=tuple(out_names),
            lowering_input_output_aliases=(),
            sim_require_finite=True,
            sim_require_nnan=True,
            nc=nc,
        )
        return tuple(outs)

    devices = jax.devices()[:n_cores]
    mesh = Mesh(np.asarray(devices), ("core",))
    in_specs = (PartitionSpec("core"),) * (n_params + n_outs)
    out_specs = (PartitionSpec("core"),) * len(out_names)
    sharded = jax.jit(
        shard_map(_body, mesh=mesh, in_specs=in_specs, out_specs=out_specs,
                  check_rep=False),
        donate_argnums=donate, keep_unused=True,
    )
    per_core = [[np.asarray(m[name]) for name in in_names[:n_params]]
                for m in in_maps]
    concat_in = [np.concatenate([per_core[c][i] for c in range(n_cores)], axis=0)
                 for i in range(n_params)]
    concat_in = jax.device_put(concat_in)
    concat_in = [jnp.asarray(a) for a in concat_in]

    def run_once():
        zeros = [np.zeros((n_cores * z.shape[0], *z.shape[1:]), z.dtype)
                 for z in zero_outs]
        outs = sharded(*concat_in, *zeros)
        jax.block_until_ready(outs)
        return [
            {name: np.asarray(outs[i]).reshape(n_cores, *out_avals[i].shape)[c]
             for i, name in enumerate(out_names)}
            for c in range(n_cores)
        ]

    return run_once


def _build_null_nc():
    """Tiny kernel used to calibrate fixed dispatch overhead."""
    import concourse.bass as bass
    import concourse.tile as tile
    import concourse.mybir as mybir

    nc = bass.Bass()
    x = nc.dram_tensor("nx", [P, 16], mybir.dt.float32, kind="ExternalInput")
    y = nc.dram_tensor("nout", [P, 16], mybir.dt.float32, kind="ExternalOutput")
    with tile.TileContext(nc) as tc:
        with tc.tile_pool(name="sb", bufs=1) as sb:
            t = sb.tile([P, 16], mybir.dt.float32, name="t", tag="t")
            nc.sync.dma_start(t[:], x[:])
            nc.sync.dma_start(y[:], t[:])
    orig = nc.to_json_bytes
    nc.to_json_bytes = lambda: _split_waits_json(orig())
    return nc


def benchmark(preds, gts, iters=30):
    """Returns (loss, per_call_times_s, null_times_s)."""
    import time

    preds = np.asarray(preds)
    gts = np.asarray(gts)
    in_maps = _prepare_in_maps(preds, gts)
    nc = _build_nc()
    run = _make_runner(nc, in_maps)

    results = run()                     # compile + first exec
    total = sum(r["out"].astype(np.float64).sum() for r in results)

    times = []
    for _ in range(iters):
        t0 = time.perf_counter()
        run()
        times.append(time.perf_counter() - t0)

    null_nc = _build_null_nc()
    null_in = [{"nx": np.zeros((P, 16), np.float32)} for _ in range(NCORES)]
    null_run = _make_runner(null_nc, null_in)
    null_run()
    null_times = []
    for _ in range(iters):
        t0 = time.perf_counter()
        null_run()
        null_times.append(time.perf_counter() - t0)

    return np.asarray(total, dtype=np.float32), times, null_times


# revision 8
# speedup vs baseline: 1.0251x; 1.0251x over previous
"""Chamfer loss kernel for Trainium2 (8 NeuronCores).

Problem: preds [8, 8192, 3], gts [8, 8192, 3] (f32).
  P[b,n,m] = ||gts[b,n] - preds[b,m]||^2
  loss = sum_{b,m} min_n P[b,n,m] + sum_{b,n} min_m P[b,n,m]

Sharding: data-parallel over batch B — one batch element per core; the host
sums the 8 per-core partial losses.

Per-core algorithm (two symmetric passes; pass A shown, pass B swaps roles):
  min_m P[n,m] = xx[n] + min_m ( -2 g[n]·p[m] + yy[m] )
The bracketed term is computed on TensorE as a K=15 bf16 matmul that is
exact to ~f32: (-2g) and p are split into bf16 hi+lo (all 4 cross terms
kept), and yy gets a 3-way bf16 split against 'ones' rows.  xx is added on
the host after the min (it is constant over the min axis).

The min-reduction drains PSUM through both PSUM-capable engines:
  - DVE reduces some 2-bank chunks directly (1 elem/lane/cyc),
  - ScalarE copies the rest to SBUF as bf16, which DVE then min-reduces via
    tensor_scalar(op0=min, op1=min, accum_out=...) at the fast packed mode.

This walrus build only accepts ONE sync-wait per instruction, so the BIR
json is post-processed to hoist extra waits onto EventSemaphore carriers.
"""

import json

import numpy as np
import ml_dtypes

BF16 = ml_dtypes.bfloat16

B, N, M, D = 8, 8192, 8192, 3
P = 128           # partitions per n-tile
NT = N // P       # 64 n-tiles
CH = 512          # one PSUM bank of f32
WIDE = 1024       # DVE/ACT operate on 2-bank chunks
NW = M // WIDE    # 8 wide chunks per n-tile
K = 18            # matmul contraction rows (split-fp32 encoding)
N_DIRECT = 3      # wide chunks per n-tile reduced directly from PSUM by DVE
NCORES = 8
BIG = 3.0e38


def _split_bf16(x):
    hi = x.astype(BF16)
    lo = (x - hi.astype(np.float32)).astype(BF16)
    return hi, lo


def _split3_bf16(x):
    """x (f32) -> three bf16 arrays summing to ~x (residual ~2^-27 rel)."""
    hi = x.astype(BF16)
    r1 = x - hi.astype(np.float32)
    mid = r1.astype(BF16)
    r2 = r1 - mid.astype(np.float32)
    lo = r2.astype(BF16)
    return hi, mid, lo


def _build_pass(a_pts, b_pts):
    """lhsT [K,N] bf16, rhs [K,M] bf16 with
    lhsT.T @ rhs ~= ||a||^2 - 2 a·b + ||b||^2  (full squared distance)."""
    a = a_pts.astype(np.float32)
    b = b_pts.astype(np.float32)
    t = -2.0 * a
    t_hi, t_lo = _split_bf16(t)
    p_hi, p_lo = _split_bf16(b)
    yy = (b * b).sum(-1, dtype=np.float32)
    yy_hi, yy_mid, yy_lo = _split3_bf16(yy)
    xx = (a * a).sum(-1, dtype=np.float32)
    xx_hi, xx_mid, xx_lo = _split3_bf16(xx)
    ones_a = np.ones((a.shape[0],), dtype=BF16)
    ones_b = np.ones((b.shape[0],), dtype=BF16)

    lhsT = np.stack(
        [t_hi[:, 0], t_hi[:, 1], t_hi[:, 2],
         t_hi[:, 0], t_hi[:, 1], t_hi[:, 2],
         t_lo[:, 0], t_lo[:, 1], t_lo[:, 2],
         t_lo[:, 0], t_lo[:, 1], t_lo[:, 2],
         ones_a, ones_a, ones_a,
         xx_hi, xx_mid, xx_lo]
    )
    rhs = np.stack(
        [p_hi[:, 0], p_hi[:, 1], p_hi[:, 2],
         p_lo[:, 0], p_lo[:, 1], p_lo[:, 2],
         p_hi[:, 0], p_hi[:, 1], p_hi[:, 2],
         p_lo[:, 0], p_lo[:, 1], p_lo[:, 2],
         yy_hi, yy_mid, yy_lo,
         ones_b, ones_b, ones_b]
    )
    return lhsT, rhs


MAX_WAITS = 1


def _split_waits_json(raw: bytes) -> bytes:
    """Hoist extra sync-waits onto EventSemaphore carriers (this walrus build
    rejects instructions with more than one wait)."""
    d = json.loads(raw)
    for f in d["functions"]:
        for blk in f["blocks"]:
            insts = blk.get("instructions")
            if not insts:
                continue
            new = []
            changed = False
            for inst in insts:
                si = inst.get("sync_info")
                waits = (si or {}).get("on_wait") or []
                if len(waits) > MAX_WAITS:
                    extra = waits[:-MAX_WAITS]
                    keep = waits[-MAX_WAITS:]
                    for k, w in enumerate(extra):
                        new.append({
                            "debug": inst.get("debug", 0),
                            "engine": inst["engine"],
                            "ins": [], "outs": [],
                            "name": f"{inst['name']}_sw{k}",
                            "opcode": "EventSemaphore",
                            "sync_info": {"on_wait": [w], "on_update": []},
                        })
                    si["on_wait"] = keep
                    changed = True
                new.append(inst)
            if changed:
                blk["instructions"] = new
    return json.dumps(d).encode()


def _build_nc():
    import concourse.bass as bass
    import concourse.tile as tile
    import concourse.mybir as mybir

    f32 = mybir.dt.float32
    bf16 = mybir.dt.bfloat16
    MIN = mybir.AluOpType.min
    X = mybir.AxisListType.X
    WT = 2048                      # wide psum tile (4 banks)
    NWT = M // WT                  # 4 wide tiles per n-tile

    nc = bass.Bass()
    dram = {}
    for nm in ("lA", "rA", "lB", "rB"):
        dram[nm] = nc.dram_tensor(nm, [K, 8192], bf16, kind="ExternalInput")
    out = nc.dram_tensor("out", [P, 2 * NT], f32, kind="ExternalOutput")

    with tile.TileContext(nc) as tc:
        with (
            tc.tile_pool(name="const", bufs=1) as cpool,
            tc.tile_pool(name="psum", bufs=2, space="PSUM") as psum,
            tc.tile_pool(name="stage", bufs=2) as stpool,
            tc.tile_pool(name="parts", bufs=2) as partpool,
        ):
            w = {}
            r = {}
            for nm in ("A", "B"):
                wt = cpool.tile([K, 8192], bf16, name=f"w{nm}", tag=f"w{nm}")
                rt = cpool.tile([K, 8192], bf16, name=f"r{nm}", tag=f"r{nm}")
                nc.sync.dma_start(wt[:, :], dram["l" + nm][:, :])
                nc.sync.dma_start(rt[:, :], dram["r" + nm][:, :])
                w[nm], r[nm] = wt, rt
            minv = cpool.tile([P, 2 * NT], f32, name="minv", tag="minv")

            for pi, nm in enumerate(("A", "B")):
                wt, rt = w[nm], r[nm]
                for i in range(NT):
                    lhsT = wt[:, P * i:P * (i + 1)]
                    n_direct = 1 if i % 2 == 0 else 2
                    n_copy = NWT - n_direct
                    parts = partpool.tile([P, 3], f32, name="parts", tag="parts")
                    stg = stpool.tile([P, 3 * WT], bf16, name="stg", tag="stg")
                    ncopied = 0
                    ndir = 0
                    for k in range(NWT):
                        W = psum.tile([P, WT], f32, name="W", tag="W")
                        for h in range(WT // CH):
                            nc.tensor.matmul(
                                W[:, CH * h:CH * (h + 1)],
                                lhsT,
                                rt[:, WT * k + CH * h:WT * k + CH * (h + 1)],
                                start=True, stop=True,
                            )
                        # direct reductions first in the rotation (k==0 and,
                        # for odd tiles, k==2) so DVE work interleaves with
                        # the ACT copies
                        if (k == 0 or (k == 2 and n_direct == 2)):
                            # first direct -> col 1, second -> col 0, so the
                            # written columns are contiguous with the ts
                            # accumulator in col 2
                            nc.vector.tensor_reduce(
                                parts[:, 1 - ndir:2 - ndir], W[:], axis=X, op=MIN,
                            )
                            ndir += 1
                        else:
                            nc.scalar.copy(
                                stg[:, ncopied * WT:(ncopied + 1) * WT], W[:])
                            ncopied += 1
                    nc.vector.tensor_scalar(
                        stg[:, :ncopied * WT], stg[:, :ncopied * WT], BIG, None,
                        op0=MIN, op1=MIN,
                        accum_out=parts[:, 2:3],
                    )
                    nc.vector.tensor_reduce(
                        minv[:, pi * NT + i:pi * NT + i + 1],
                        parts[:, 2 - ndir:3],
                        axis=X, op=MIN,
                    )
            nc.sync.dma_start(out[:, :], minv[:])

    orig = nc.to_json_bytes
    nc.to_json_bytes = lambda: _split_waits_json(orig())
    return nc


_LAST_RESULTS = None


def _prepare_in_maps(preds, gts):
    in_maps = []
    for b in range(B):
        lA, rA = _build_pass(gts[b], preds[b])     # min over preds per gt
        lB, rB = _build_pass(preds[b], gts[b])     # min over gts per pred
        in_maps.append({"lA": lA, "rA": rA, "lB": lB, "rB": rB})
    return in_maps


def kernel(preds, gts, _trace=False):
    from concourse.bass_utils import run_bass_kernel_spmd

    global _LAST_RESULTS
    preds = np.asarray(preds)
    gts = np.asarray(gts)
    assert preds.shape == (B, M, D) and gts.shape == (B, N, D)

    in_maps = _prepare_in_maps(preds, gts)
    nc = _build_nc()
    res = run_bass_kernel_spmd(
        nc, in_maps, core_ids=list(range(NCORES)), trace=_trace,
    )
    _LAST_RESULTS = res

    total = 0.0
    for b in range(B):
        total += res.results[b]["out"].astype(np.float64).sum()
    return np.asarray(total, dtype=np.float32)


# ----------------------------------------------------------------------------
# Benchmark support (test-only): build the jitted sharded executable once and
# re-invoke it, so per-call wall time ~= dispatch overhead + NEFF exec time.
# ----------------------------------------------------------------------------

def _make_runner(nc, in_maps):
    import jax
    import jax.numpy as jnp
    import concourse.mybir as mybir
    from concourse import bass2jax
    from jax.experimental.shard_map import shard_map
    from jax.sharding import Mesh, PartitionSpec

    bass2jax.install_neuronx_cc_hook()
    n_cores = len(in_maps)

    partition_name = nc.partition_id_tensor.name if nc.partition_id_tensor else None
    in_names, out_names, out_avals, zero_outs = [], [], [], []
    for alloc in nc.m.functions[0].allocations:
        if not isinstance(alloc, mybir.MemoryLocationSet):
            continue
        name = alloc.memorylocations[0].name
        if alloc.kind == "ExternalInput":
            if name != partition_name:
                in_names.append(name)
        elif alloc.kind == "ExternalOutput":
            shape = tuple(alloc.tensor_shape)
            dtype = mybir.dt.np(alloc.dtype)
            out_names.append(name)
            out_avals.append(jax.core.ShapedArray(shape, dtype))
            zero_outs.append(np.zeros(shape, dtype))
    n_params = len(in_names)
    n_outs = len(out_avals)
    in_names = in_names + out_names
    if partition_name is not None:
        in_names.append(partition_name)
    donate = tuple(range(n_params, n_params + n_outs))

    def _body(*args):
        operands = list(args)
        if partition_name is not None:
            operands.append(bass2jax.partition_id_tensor())
        outs = bass2jax._bass_exec_p.bind(
            *operands,
            out_avals=tuple(out_avals),
            in_names=tuple(in_names),
            out_names=tuple(out_names),
            lowering_input_output_aliases=(),
            sim_require_finite=True,
            sim_require_nnan=True,
            nc=nc,
        )
        return tuple(outs)

    devices = jax.devices()[:n_cores]
    mesh = Mesh(np.asarray(devices), ("core",))
    in_specs = (PartitionSpec("core"),) * (n_params + n_outs)
    out_specs = (PartitionSpec("core"),) * len(out_names)
    sharded = jax.jit(
        shard_map(_body, mesh=mesh, in_specs=in_specs, out_specs=out_specs,
                  check_rep=False),
        donate_argnums=donate, keep_unused=True,
    )
    per_core = [[np.asarray(m[name]) for name in in_names[:n_params]]
                for m in in_maps]
    concat_in = [np.concatenate([per_core[c][i] for c in range(n_cores)], axis=0)
                 for i in range(n_params)]
    concat_in = jax.device_put(concat_in)
    concat_in = [jnp.asarray(a) for a in concat_in]

    def run_once():
        zeros = [np.zeros((n_cores * z.shape[0], *z.shape[1:]), z.dtype)
                 for z in zero_outs]
        outs = sharded(*concat_in, *zeros)
        jax.block_until_ready(outs)
        return [
            {name: np.asarray(outs[i]).reshape(n_cores, *out_avals[i].shape)[c]
             for i, name in enumerate(out_names)}
            for c in range(n_cores)
        ]

    return run_once


def _build_null_nc():
    """Tiny kernel used to calibrate fixed dispatch overhead."""
    import concourse.bass as bass
    import concourse.tile as tile
    import concourse.mybir as mybir

    nc = bass.Bass()
    x = nc.dram_tensor("nx", [P, 16], mybir.dt.float32, kind="ExternalInput")
    y = nc.dram_tensor("nout", [P, 16], mybir.dt.float32, kind="ExternalOutput")
    with tile.TileContext(nc) as tc:
        with tc.tile_pool(name="sb", bufs=1) as sb:
            t = sb.tile([P, 16], mybir.dt.float32, name="t", tag="t")
            nc.sync.dma_start(t[:], x[:])
            nc.sync.dma_start(y[:], t[:])
    orig = nc.to_json_bytes
    nc.to_json_bytes = lambda: _split_waits_json(orig())
    return nc


def benchmark(preds, gts, iters=30):
    """Returns (loss, per_call_times_s, null_times_s)."""
    import time

    preds = np.asarray(preds)
    gts = np.asarray(gts)
    in_maps = _prepare_in_maps(preds, gts)
    nc = _build_nc()
    run = _make_runner(nc, in_maps)

    results = run()                     # compile + first exec
    total = sum(r["out"].astype(np.float64).sum() for r in results)

    times = []
    for _ in range(iters):
        t0 = time.perf_counter()
        run()
        times.append(time.perf_counter() - t0)

    null_nc = _build_null_nc()
    null_in = [{"nx": np.zeros((P, 16), np.float32)} for _ in range(NCORES)]
    null_run = _make_runner(null_nc, null_in)
    null_run()
    null_times = []
    for _ in range(iters):
        t0 = time.perf_counter()
        null_run()
        null_times.append(time.perf_counter() - t0)

    return np.asarray(total, dtype=np.float32), times, null_times


# revision 13
# speedup vs baseline: 1.1719x; 1.1432x over previous
"""Chamfer loss kernel for Trainium2 (8 NeuronCores).

Problem: preds [8, 8192, 3], gts [8, 8192, 3] (f32).
  P[b,n,m] = ||gts[b,n] - preds[b,m]||^2
  loss = sum_{b,m} min_n P[b,n,m] + sum_{b,n} min_m P[b,n,m]

Sharding: data-parallel over batch B — one batch element per core; the host
sums the 8 per-core partial losses.

Per-core algorithm (two symmetric passes; pass A shown, pass B swaps roles):
  min_m P[n,m] = xx[n] + min_m ( -2 g[n]·p[m] + yy[m] )
The bracketed term is computed on TensorE as a K=15 bf16 matmul that is
exact to ~f32: (-2g) and p are split into bf16 hi+lo (all 4 cross terms
kept), and yy gets a 3-way bf16 split against 'ones' rows.  xx is added on
the host after the min (it is constant over the min axis).

The min-reduction drains PSUM through both PSUM-capable engines:
  - DVE reduces some 2-bank chunks directly (1 elem/lane/cyc),
  - ScalarE copies the rest to SBUF as bf16, which DVE then min-reduces via
    tensor_scalar(op0=min, op1=min, accum_out=...) at the fast packed mode.

This walrus build only accepts ONE sync-wait per instruction, so the BIR
json is post-processed to hoist extra waits onto EventSemaphore carriers.
"""

import json

import numpy as np
import ml_dtypes

BF16 = ml_dtypes.bfloat16

B, N, M, D = 8, 8192, 8192, 3
P = 128           # partitions per n-tile
NT = N // P       # 64 n-tiles
CH = 512          # one PSUM bank of f32
WIDE = 1024       # DVE/ACT operate on 2-bank chunks
NW = M // WIDE    # 8 wide chunks per n-tile
K = 18            # matmul contraction rows (split-fp32 encoding)
N_DIRECT = 3      # wide chunks per n-tile reduced directly from PSUM by DVE
NCORES = 8
BIG = 3.0e38


def _split_bf16(x):
    hi = x.astype(BF16)
    lo = (x - hi.astype(np.float32)).astype(BF16)
    return hi, lo


def _split3_bf16(x):
    """x (f32) -> three bf16 arrays summing to ~x (residual ~2^-27 rel)."""
    hi = x.astype(BF16)
    r1 = x - hi.astype(np.float32)
    mid = r1.astype(BF16)
    r2 = r1 - mid.astype(np.float32)
    lo = r2.astype(BF16)
    return hi, mid, lo


def _build_pass(a_pts, b_pts):
    """lhsT [K,N] bf16, rhs [K,M] bf16 with
    lhsT.T @ rhs ~= ||a||^2 - 2 a·b + ||b||^2  (full squared distance)."""
    a = a_pts.astype(np.float32)
    b = b_pts.astype(np.float32)
    t = -2.0 * a
    t_hi, t_lo = _split_bf16(t)
    p_hi, p_lo = _split_bf16(b)
    yy = (b * b).sum(-1, dtype=np.float32)
    yy_hi, yy_mid, yy_lo = _split3_bf16(yy)
    xx = (a * a).sum(-1, dtype=np.float32)
    xx_hi, xx_mid, xx_lo = _split3_bf16(xx)
    ones_a = np.ones((a.shape[0],), dtype=BF16)
    ones_b = np.ones((b.shape[0],), dtype=BF16)

    lhsT = np.stack(
        [t_hi[:, 0], t_hi[:, 1], t_hi[:, 2],
         t_hi[:, 0], t_hi[:, 1], t_hi[:, 2],
         t_lo[:, 0], t_lo[:, 1], t_lo[:, 2],
         t_lo[:, 0], t_lo[:, 1], t_lo[:, 2],
         ones_a, ones_a, ones_a,
         xx_hi, xx_mid, xx_lo]
    )
    rhs = np.stack(
        [p_hi[:, 0], p_hi[:, 1], p_hi[:, 2],
         p_lo[:, 0], p_lo[:, 1], p_lo[:, 2],
         p_hi[:, 0], p_hi[:, 1], p_hi[:, 2],
         p_lo[:, 0], p_lo[:, 1], p_lo[:, 2],
         yy_hi, yy_mid, yy_lo,
         ones_b, ones_b, ones_b]
    )
    return lhsT, rhs


MAX_WAITS = 1


def _split_waits_json(raw: bytes) -> bytes:
    """Hoist extra sync-waits onto EventSemaphore carriers (this walrus build
    rejects instructions with more than one wait)."""
    d = json.loads(raw)
    for f in d["functions"]:
        for blk in f["blocks"]:
            insts = blk.get("instructions")
            if not insts:
                continue
            new = []
            changed = False
            for inst in insts:
                si = inst.get("sync_info")
                waits = (si or {}).get("on_wait") or []
                if len(waits) > MAX_WAITS:
                    extra = waits[:-MAX_WAITS]
                    keep = waits[-MAX_WAITS:]
                    for k, w in enumerate(extra):
                        new.append({
                            "debug": inst.get("debug", 0),
                            "engine": inst["engine"],
                            "ins": [], "outs": [],
                            "name": f"{inst['name']}_sw{k}",
                            "opcode": "EventSemaphore",
                            "sync_info": {"on_wait": [w], "on_update": []},
                        })
                    si["on_wait"] = keep
                    changed = True
                new.append(inst)
            if changed:
                blk["instructions"] = new
    return json.dumps(d).encode()


def _build_nc():
    import concourse.bass as bass
    import concourse.tile as tile
    import concourse.mybir as mybir

    f32 = mybir.dt.float32
    bf16 = mybir.dt.bfloat16
    MIN = mybir.AluOpType.min
    X = mybir.AxisListType.X
    WT = 1024                      # psum tile (2 banks); 4 pool slots = 8 banks
    NWT = M // WT                  # 8 tiles per n-tile
    ND = 3                         # direct DVE reductions per n-tile
    NC_ = NWT - ND                 # ACT-copied chunks per n-tile

    nc = bass.Bass()
    dram = {}
    for nm in ("lA", "rA", "lB", "rB"):
        dram[nm] = nc.dram_tensor(nm, [K, 8192], bf16, kind="ExternalInput")
    out = nc.dram_tensor("out", [P, 2 * NT], f32, kind="ExternalOutput")

    with tile.TileContext(nc) as tc:
        with (
            tc.tile_pool(name="const", bufs=1) as cpool,
            tc.tile_pool(name="psum", bufs=4, space="PSUM") as psum,
            tc.tile_pool(name="stage", bufs=2) as stpool,
            tc.tile_pool(name="parts", bufs=2) as partpool,
        ):
            w = {}
            r = {}
            for nm in ("A", "B"):
                wt = cpool.tile([K, 8192], bf16, name=f"w{nm}", tag=f"w{nm}")
                rt = cpool.tile([K, 8192], bf16, name=f"r{nm}", tag=f"r{nm}")
                nc.sync.dma_start(wt[:, :], dram["l" + nm][:, :])
                nc.sync.dma_start(rt[:, :], dram["r" + nm][:, :])
                w[nm], r[nm] = wt, rt
            minv = cpool.tile([P, 2 * NT], f32, name="minv", tag="minv")

            GRP = 4                # n-tiles whose partials merge in one reduce
            NP = ND + 1            # partial-min columns per n-tile
            for pi, nm in enumerate(("A", "B")):
                wt, rt = w[nm], r[nm]
                for g in range(NT // GRP):
                    parts = partpool.tile([P, GRP * NP], f32,
                                          name="parts", tag="parts")
                    for j in range(GRP):
                        i = g * GRP + j
                        lhsT = wt[:, P * i:P * (i + 1)]
                        stg = stpool.tile([P, NC_ * WT], bf16,
                                          name="stg", tag="stg")
                        ncopied = 0
                        ndir = 0
                        for k in range(NWT):
                            W = psum.tile([P, WT], f32, name="W", tag="W")
                            for h in range(WT // CH):
                                nc.tensor.matmul(
                                    W[:, CH * h:CH * (h + 1)],
                                    lhsT,
                                    rt[:, WT * k + CH * h:WT * k + CH * (h + 1)],
                                    start=True, stop=True,
                                )
                            # ACT copies first (k 0..NC_-1), then the bf16 min
                            # over the staged block, then the direct DVE
                            # reductions — so the ts never trails the tile.
                            if k < NC_:
                                nc.scalar.copy(
                                    stg[:, ncopied * WT:(ncopied + 1) * WT], W[:])
                                ncopied += 1
                                if ncopied == NC_:
                                    nc.vector.tensor_scalar(
                                        stg[:], stg[:], BIG, None,
                                        op0=MIN, op1=MIN,
                                        accum_out=parts[:, j * NP + ND:
                                                        j * NP + ND + 1],
                                    )
                            else:
                                nc.vector.tensor_reduce(
                                    parts[:, j * NP + ndir:j * NP + ndir + 1],
                                    W[:], axis=X, op=MIN,
                                )
                                ndir += 1
                    nc.vector.tensor_reduce(
                        minv[:, pi * NT + g * GRP:pi * NT + (g + 1) * GRP],
                        parts[:].rearrange("p (g n) -> p g n", n=NP),
                        axis=X, op=MIN,
                    )
            nc.sync.dma_start(out[:, :], minv[:])

    orig = nc.to_json_bytes
    nc.to_json_bytes = lambda: _split_waits_json(orig())
    return nc


_LAST_RESULTS = None


def _prepare_in_maps(preds, gts):
    in_maps = []
    for b in range(B):
        lA, rA = _build_pass(gts[b], preds[b])     # min over preds per gt
        lB, rB = _build_pass(preds[b], gts[b])     # min over gts per pred
        in_maps.append({"lA": lA, "rA": rA, "lB": lB, "rB": rB})
    return in_maps


def kernel(preds, gts, _trace=False):
    from concourse.bass_utils import run_bass_kernel_spmd

    global _LAST_RESULTS
    preds = np.asarray(preds)
    gts = np.asarray(gts)
    assert preds.shape == (B, M, D) and gts.shape == (B, N, D)

    in_maps = _prepare_in_maps(preds, gts)
    nc = _build_nc()
    res = run_bass_kernel_spmd(
        nc, in_maps, core_ids=list(range(NCORES)), trace=_trace,
    )
    _LAST_RESULTS = res

    total = 0.0
    for b in range(B):
        total += res.results[b]["out"].astype(np.float64).sum()
    return np.asarray(total, dtype=np.float32)


# ----------------------------------------------------------------------------
# Benchmark support (test-only): build the jitted sharded executable once and
# re-invoke it, so per-call wall time ~= dispatch overhead + NEFF exec time.
# ----------------------------------------------------------------------------

def _make_runner(nc, in_maps):
    import jax
    import jax.numpy as jnp
    import concourse.mybir as mybir
    from concourse import bass2jax
    from jax.experimental.shard_map import shard_map
    from jax.sharding import Mesh, PartitionSpec

    bass2jax.install_neuronx_cc_hook()
    n_cores = len(in_maps)

    partition_name = nc.partition_id_tensor.name if nc.partition_id_tensor else None
    in_names, out_names, out_avals, zero_outs = [], [], [], []
    for alloc in nc.m.functions[0].allocations:
        if not isinstance(alloc, mybir.MemoryLocationSet):
            continue
        name = alloc.memorylocations[0].name
        if alloc.kind == "ExternalInput":
            if name != partition_name:
                in_names.append(name)
        elif alloc.kind == "ExternalOutput":
            shape = tuple(alloc.tensor_shape)
            dtype = mybir.dt.np(alloc.dtype)
            out_names.append(name)
            out_avals.append(jax.core.ShapedArray(shape, dtype))
            zero_outs.append(np.zeros(shape, dtype))
    n_params = len(in_names)
    n_outs = len(out_avals)
    in_names = in_names + out_names
    if partition_name is not None:
        in_names.append(partition_name)
    donate = tuple(range(n_params, n_params + n_outs))

    def _body(*args):
        operands = list(args)
        if partition_name is not None:
            operands.append(bass2jax.partition_id_tensor())
        outs = bass2jax._bass_exec_p.bind(
            *operands,
            out_avals=tuple(out_avals),
            in_names=tuple(in_names),
            out_names=tuple(out_names),
            lowering_input_output_aliases=(),
            sim_require_finite=True,
            sim_require_nnan=True,
            nc=nc,
        )
        return tuple(outs)

    devices = jax.devices()[:n_cores]
    mesh = Mesh(np.asarray(devices), ("core",))
    in_specs = (PartitionSpec("core"),) * (n_params + n_outs)
    out_specs = (PartitionSpec("core"),) * len(out_names)
    sharded = jax.jit(
        shard_map(_body, mesh=mesh, in_specs=in_specs, out_specs=out_specs,
                  check_rep=False),
        donate_argnums=donate, keep_unused=True,
    )
    per_core = [[np.asarray(m[name]) for name in in_names[:n_params]]
                for m in in_maps]
    concat_in = [np.concatenate([per_core[c][i] for c in range(n_cores)], axis=0)
                 for i in range(n_params)]
    concat_in = jax.device_put(concat_in)
    concat_in = [jnp.asarray(a) for a in concat_in]

    def run_once():
        zeros = [np.zeros((n_cores * z.shape[0], *z.shape[1:]), z.dtype)
                 for z in zero_outs]
        outs = sharded(*concat_in, *zeros)
        jax.block_until_ready(outs)
        return [
            {name: np.asarray(outs[i]).reshape(n_cores, *out_avals[i].shape)[c]
             for i, name in enumerate(out_names)}
            for c in range(n_cores)
        ]

    return run_once


def _build_null_nc():
    """Tiny kernel used to calibrate fixed dispatch overhead."""
    import concourse.bass as bass
    import concourse.tile as tile
    import concourse.mybir as mybir

    nc = bass.Bass()
    x = nc.dram_tensor("nx", [P, 16], mybir.dt.float32, kind="ExternalInput")
    y = nc.dram_tensor("nout", [P, 16], mybir.dt.float32, kind="ExternalOutput")
    with tile.TileContext(nc) as tc:
        with tc.tile_pool(name="sb", bufs=1) as sb:
            t = sb.tile([P, 16], mybir.dt.float32, name="t", tag="t")
            nc.sync.dma_start(t[:], x[:])
            nc.sync.dma_start(y[:], t[:])
    orig = nc.to_json_bytes
    nc.to_json_bytes = lambda: _split_waits_json(orig())
    return nc


def benchmark(preds, gts, iters=30):
    """Returns (loss, per_call_times_s, null_times_s)."""
    import time

    preds = np.asarray(preds)
    gts = np.asarray(gts)
    in_maps = _prepare_in_maps(preds, gts)
    nc = _build_nc()
    run = _make_runner(nc, in_maps)

    results = run()                     # compile + first exec
    total = sum(r["out"].astype(np.float64).sum() for r in results)

    times = []
    for _ in range(iters):
        t0 = time.perf_counter()
        run()
        times.append(time.perf_counter() - t0)

    null_nc = _build_null_nc()
    null_in = [{"nx": np.zeros((P, 16), np.float32)} for _ in range(NCORES)]
    null_run = _make_runner(null_nc, null_in)
    null_run()
    null_times = []
    for _ in range(iters):
        t0 = time.perf_counter()
        null_run()
        null_times.append(time.perf_counter() - t0)

    return np.asarray(total, dtype=np.float32), times, null_times


# revision 14
# speedup vs baseline: 30.6210x; 26.1295x over previous
"""Chamfer loss kernel for Trainium2 (8 NeuronCores).

Problem: preds [8, 8192, 3], gts [8, 8192, 3] (f32).
  P[b,n,m] = ||gts[b,n] - preds[b,m]||^2
  loss = sum_{b,m} min_n P[b,n,m] + sum_{b,n} min_m P[b,n,m]

Sharding: data-parallel over batch B — one batch element per core; the host
sums the 8 per-core partial losses.

Per-core algorithm (two symmetric passes; pass A shown, pass B swaps roles):
  min_m P[n,m] = xx[n] + min_m ( -2 g[n]·p[m] + yy[m] )
The bracketed term is computed on TensorE as a K=15 bf16 matmul that is
exact to ~f32: (-2g) and p are split into bf16 hi+lo (all 4 cross terms
kept), and yy gets a 3-way bf16 split against 'ones' rows.  xx is added on
the host after the min (it is constant over the min axis).

The min-reduction drains PSUM through both PSUM-capable engines:
  - DVE reduces some 2-bank chunks directly (1 elem/lane/cyc),
  - ScalarE copies the rest to SBUF as bf16, which DVE then min-reduces via
    tensor_scalar(op0=min, op1=min, accum_out=...) at the fast packed mode.

This walrus build only accepts ONE sync-wait per instruction, so the BIR
json is post-processed to hoist extra waits onto EventSemaphore carriers.
"""

import json

import numpy as np
import ml_dtypes

BF16 = ml_dtypes.bfloat16

B, N, M, D = 8, 8192, 8192, 3
P = 128           # partitions per n-tile
NT = N // P       # 64 n-tiles
CH = 512          # one PSUM bank of f32
WIDE = 1024       # DVE/ACT operate on 2-bank chunks
NW = M // WIDE    # 8 wide chunks per n-tile
K = 18            # matmul contraction rows (split-fp32 encoding)
N_DIRECT = 3      # wide chunks per n-tile reduced directly from PSUM by DVE
NCORES = 8
BIG = 3.0e38


def _split_bf16(x):
    hi = x.astype(BF16)
    lo = (x - hi.astype(np.float32)).astype(BF16)
    return hi, lo


def _split3_bf16(x):
    """x (f32) -> three bf16 arrays summing to ~x (residual ~2^-27 rel)."""
    hi = x.astype(BF16)
    r1 = x - hi.astype(np.float32)
    mid = r1.astype(BF16)
    r2 = r1 - mid.astype(np.float32)
    lo = r2.astype(BF16)
    return hi, mid, lo


def _build_pass(a_pts, b_pts):
    """lhsT [K,N] bf16, rhs [K,M] bf16 with
    lhsT.T @ rhs ~= ||a||^2 - 2 a·b + ||b||^2  (full squared distance)."""
    a = a_pts.astype(np.float32)
    b = b_pts.astype(np.float32)
    t = -2.0 * a
    t_hi, t_lo = _split_bf16(t)
    p_hi, p_lo = _split_bf16(b)
    yy = (b * b).sum(-1, dtype=np.float32)
    yy_hi, yy_mid, yy_lo = _split3_bf16(yy)
    xx = (a * a).sum(-1, dtype=np.float32)
    xx_hi, xx_mid, xx_lo = _split3_bf16(xx)
    ones_a = np.ones((a.shape[0],), dtype=BF16)
    ones_b = np.ones((b.shape[0],), dtype=BF16)

    lhsT = np.stack(
        [t_hi[:, 0], t_hi[:, 1], t_hi[:, 2],
         t_hi[:, 0], t_hi[:, 1], t_hi[:, 2],
         t_lo[:, 0], t_lo[:, 1], t_lo[:, 2],
         t_lo[:, 0], t_lo[:, 1], t_lo[:, 2],
         ones_a, ones_a, ones_a,
         xx_hi, xx_mid, xx_lo]
    )
    rhs = np.stack(
        [p_hi[:, 0], p_hi[:, 1], p_hi[:, 2],
         p_lo[:, 0], p_lo[:, 1], p_lo[:, 2],
         p_hi[:, 0], p_hi[:, 1], p_hi[:, 2],
         p_lo[:, 0], p_lo[:, 1], p_lo[:, 2],
         yy_hi, yy_mid, yy_lo,
         ones_b, ones_b, ones_b]
    )
    return lhsT, rhs


MAX_WAITS = 1


def _split_waits_json(raw: bytes) -> bytes:
    """Hoist extra sync-waits onto EventSemaphore carriers (this walrus build
    rejects instructions with more than one wait)."""
    d = json.loads(raw)
    for f in d["functions"]:
        for blk in f["blocks"]:
            insts = blk.get("instructions")
            if not insts:
                continue
            new = []
            changed = False
            for inst in insts:
                si = inst.get("sync_info")
                waits = (si or {}).get("on_wait") or []
                if len(waits) > MAX_WAITS:
                    extra = waits[:-MAX_WAITS]
                    keep = waits[-MAX_WAITS:]
                    for k, w in enumerate(extra):
                        new.append({
                            "debug": inst.get("debug", 0),
                            "engine": inst["engine"],
                            "ins": [], "outs": [],
                            "name": f"{inst['name']}_sw{k}",
                            "opcode": "EventSemaphore",
                            "sync_info": {"on_wait": [w], "on_update": []},
                        })
                    si["on_wait"] = keep
                    changed = True
                new.append(inst)
            if changed:
                blk["instructions"] = new
    return json.dumps(d).encode()


def _build_nc():
    import concourse.bass as bass
    import concourse.tile as tile
    import concourse.mybir as mybir

    f32 = mybir.dt.float32
    bf16 = mybir.dt.bfloat16
    MIN = mybir.AluOpType.min
    X = mybir.AxisListType.X
    WT = 1024                      # psum tile (2 banks); 4 pool slots = 8 banks
    NWT = M // WT                  # 8 tiles per n-tile
    ND = 3                         # direct DVE reductions per n-tile
    NC_ = NWT - ND                 # ACT-copied chunks per n-tile

    nc = bass.Bass()
    dram = {}
    for nm in ("lA", "rA", "lB", "rB"):
        dram[nm] = nc.dram_tensor(nm, [K, 8192], bf16, kind="ExternalInput")
    out = nc.dram_tensor("out", [P, 2 * NT], f32, kind="ExternalOutput")

    with tile.TileContext(nc) as tc:
        with (
            tc.tile_pool(name="const", bufs=1) as cpool,
            tc.tile_pool(name="psum", bufs=4, space="PSUM") as psum,
            tc.tile_pool(name="stage", bufs=2) as stpool,
            tc.tile_pool(name="parts", bufs=2) as partpool,
        ):
            w = {}
            r = {}
            for nm in ("A", "B"):
                wt = cpool.tile([K, 8192], bf16, name=f"w{nm}", tag=f"w{nm}")
                rt = cpool.tile([K, 8192], bf16, name=f"r{nm}", tag=f"r{nm}")
                # chunked loads so the first matmuls start early
                for c in range(4):
                    s = slice(2048 * c, 2048 * (c + 1))
                    nc.sync.dma_start(wt[:, s], dram["l" + nm][:, s])
                    nc.sync.dma_start(rt[:, s], dram["r" + nm][:, s])
                w[nm], r[nm] = wt, rt
            minv = cpool.tile([P, 2 * NT], f32, name="minv", tag="minv")

            GRP = 4                # n-tiles whose partials merge in one reduce
            NP = ND + 1            # partial-min columns per n-tile
            for pi, nm in enumerate(("A", "B")):
                wt, rt = w[nm], r[nm]
                for g in range(NT // GRP):
                    parts = partpool.tile([P, GRP * NP], f32,
                                          name="parts", tag="parts")
                    for j in range(GRP):
                        i = g * GRP + j
                        lhsT = wt[:, P * i:P * (i + 1)]
                        stg = stpool.tile([P, NC_ * WT], bf16,
                                          name="stg", tag="stg")
                        ncopied = 0
                        ndir = 0
                        for k in range(NWT):
                            W = psum.tile([P, WT], f32, name="W", tag="W")
                            for h in range(WT // CH):
                                nc.tensor.matmul(
                                    W[:, CH * h:CH * (h + 1)],
                                    lhsT,
                                    rt[:, WT * k + CH * h:WT * k + CH * (h + 1)],
                                    start=True, stop=True,
                                )
                            # ACT copies first (k 0..NC_-1), then the bf16 min
                            # over the staged block, then the direct DVE
                            # reductions — so the ts never trails the tile.
                            if k < NC_:
                                nc.scalar.copy(
                                    stg[:, ncopied * WT:(ncopied + 1) * WT], W[:])
                                ncopied += 1
                                if ncopied == NC_:
                                    nc.vector.tensor_scalar(
                                        stg[:], stg[:], BIG, None,
                                        op0=MIN, op1=MIN,
                                        accum_out=parts[:, j * NP + ND:
                                                        j * NP + ND + 1],
                                    )
                            else:
                                nc.vector.tensor_reduce(
                                    parts[:, j * NP + ndir:j * NP + ndir + 1],
                                    W[:], axis=X, op=MIN,
                                )
                                ndir += 1
                    nc.vector.tensor_reduce(
                        minv[:, pi * NT + g * GRP:pi * NT + (g + 1) * GRP],
                        parts[:].rearrange("p (g n) -> p g n", n=NP),
                        axis=X, op=MIN,
                    )
            nc.sync.dma_start(out[:, :], minv[:])

    orig = nc.to_json_bytes
    nc.to_json_bytes = lambda: _split_waits_json(orig())
    return nc


_LAST_RESULTS = None


def _prepare_in_maps(preds, gts):
    in_maps = []
    for b in range(B):
        lA, rA = _build_pass(gts[b], preds[b])     # min over preds per gt
        lB, rB = _build_pass(preds[b], gts[b])     # min over gts per pred
        in_maps.append({"lA": lA, "rA": rA, "lB": lB, "rB": rB})
    return in_maps


def kernel(preds, gts, _trace=False):
    from concourse.bass_utils import run_bass_kernel_spmd

    global _LAST_RESULTS
    preds = np.asarray(preds)
    gts = np.asarray(gts)
    assert preds.shape == (B, M, D) and gts.shape == (B, N, D)

    in_maps = _prepare_in_maps(preds, gts)
    nc = _build_nc()
    res = run_bass_kernel_spmd(
        nc, in_maps, core_ids=list(range(NCORES)), trace=_trace,
    )
    _LAST_RESULTS = res

    total = 0.0
    for b in range(B):
        total += res.results[b]["out"].astype(np.float64).sum()
    return np.asarray(total, dtype=np.float32)


# ----------------------------------------------------------------------------
# Benchmark support (test-only): build the jitted sharded executable once and
# re-invoke it, so per-call wall time ~= dispatch overhead + NEFF exec time.
# ----------------------------------------------------------------------------

def _make_runner(nc, in_maps):
    import jax
    import jax.numpy as jnp
    import concourse.mybir as mybir
    from concourse import bass2jax
    from jax.experimental.shard_map import shard_map
    from jax.sharding import Mesh, PartitionSpec

    bass2jax.install_neuronx_cc_hook()
    n_cores = len(in_maps)

    partition_name = nc.partition_id_tensor.name if nc.partition_id_tensor else None
    in_names, out_names, out_avals, zero_outs = [], [], [], []
    for alloc in nc.m.functions[0].allocations:
        if not isinstance(alloc, mybir.MemoryLocationSet):
            continue
        name = alloc.memorylocations[0].name
        if alloc.kind == "ExternalInput":
            if name != partition_name:
                in_names.append(name)
        elif alloc.kind == "ExternalOutput":
            shape = tuple(alloc.tensor_shape)
            dtype = mybir.dt.np(alloc.dtype)
            out_names.append(name)
            out_avals.append(jax.core.ShapedArray(shape, dtype))
            zero_outs.append(np.zeros(shape, dtype))
    n_params = len(in_names)
    n_outs = len(out_avals)
    in_names = in_names + out_names
    if partition_name is not None:
        in_names.append(partition_name)
    donate = tuple(range(n_params, n_params + n_outs))

    def _body(*args):
        operands = list(args)
        if partition_name is not None:
            operands.append(bass2jax.partition_id_tensor())
        outs = bass2jax._bass_exec_p.bind(
            *operands,
            out_avals=tuple(out_avals),
            in_names=tuple(in_names),
            out_names=tuple(out_names),
            lowering_input_output_aliases=(),
            sim_require_finite=True,
            sim_require_nnan=True,
            nc=nc,
        )
        return tuple(outs)

    devices = jax.devices()[:n_cores]
    mesh = Mesh(np.asarray(devices), ("core",))
    in_specs = (PartitionSpec("core"),) * (n_params + n_outs)
    out_specs = (PartitionSpec("core"),) * len(out_names)
    sharded = jax.jit(
        shard_map(_body, mesh=mesh, in_specs=in_specs, out_specs=out_specs,
                  check_rep=False),
        donate_argnums=donate, keep_unused=True,
    )
    per_core = [[np.asarray(m[name]) for name in in_names[:n_params]]
                for m in in_maps]
    concat_in = [np.concatenate([per_core[c][i] for c in range(n_cores)], axis=0)
                 for i in range(n_params)]
    concat_in = jax.device_put(concat_in)
    concat_in = [jnp.asarray(a) for a in concat_in]

    def run_once():
        zeros = [np.zeros((n_cores * z.shape[0], *z.shape[1:]), z.dtype)
                 for z in zero_outs]
        outs = sharded(*concat_in, *zeros)
        jax.block_until_ready(outs)
        return [
            {name: np.asarray(outs[i]).reshape(n_cores, *out_avals[i].shape)[c]
             for i, name in enumerate(out_names)}
            for c in range(n_cores)
        ]

    return run_once


def _build_null_nc():
    """Tiny kernel used to calibrate fixed dispatch overhead."""
    import concourse.bass as bass
    import concourse.tile as tile
    import concourse.mybir as mybir

    nc = bass.Bass()
    x = nc.dram_tensor("nx", [P, 16], mybir.dt.float32, kind="ExternalInput")
    y = nc.dram_tensor("nout", [P, 16], mybir.dt.float32, kind="ExternalOutput")
    with tile.TileContext(nc) as tc:
        with tc.tile_pool(name="sb", bufs=1) as sb:
            t = sb.tile([P, 16], mybir.dt.float32, name="t", tag="t")
            nc.sync.dma_start(t[:], x[:])
            nc.sync.dma_start(y[:], t[:])
    orig = nc.to_json_bytes
    nc.to_json_bytes = lambda: _split_waits_json(orig())
    return nc


def benchmark(preds, gts, iters=30):
    """Returns (loss, per_call_times_s, null_times_s)."""
    import time

    preds = np.asarray(preds)
    gts = np.asarray(gts)
    in_maps = _prepare_in_maps(preds, gts)
    nc = _build_nc()
    run = _make_runner(nc, in_maps)

    results = run()                     # compile + first exec
    total = sum(r["out"].astype(np.float64).sum() for r in results)

    times = []
    for _ in range(iters):
        t0 = time.perf_counter()
        run()
        times.append(time.perf_counter() - t0)

    null_nc = _build_null_nc()
    null_in = [{"nx": np.zeros((P, 16), np.float32)} for _ in range(NCORES)]
    null_run = _make_runner(null_nc, null_in)
    null_run()
    null_times = []
    for _ in range(iters):
        t0 = time.perf_counter()
        null_run()
        null_times.append(time.perf_counter() - t0)

    return np.asarray(total, dtype=np.float32), times, null_times
